# revision 33
# baseline (speedup 1.0000x reference)
"""Trainium2 8-core kernel for nn_Attention_88948772700322.

Reference computes (N=1024, B=4, C=1024, H=16, hd=64):
    qkv = x @ w_qkv.T                      [N,B,3C]
    q,k,v per (b,h); attn = softmax(q k^T / 8) v
    out = (attn.transpose(2,1,0,3)).reshape(N,B,C) @ w_proj.T + b_proj
The reshape interleaves H and B: proj-input channel c of output-batch bn is
attention head h = 4*bn + c//256, original batch b2 = (c%256)//64, dim d = c%64.

Sharding: tensor-parallel over heads — core i owns heads {2i, 2i+1}, all
batches/tokens (6.44 GFLOP/core, perfectly balanced).  Each core computes a
partial projection over its 512 proj-input channels for output batch bn=i//2;
host sums core pairs (the "all-reduce after proj" realized in unshard).

Host-side prep absorbs every layout nuisance:
  - xT [C, B*N] bf16, tokens batch-major  -> qkv needs no on-chip transpose
  - w_qk [C, 256] (cols q_h0,q_h1,k_h0,k_h1), q pre-scaled by 1/8
  - w_v  [C, 128] (cols v_h0,v_h1)
  - w_p  [512, 1024] = w_proj columns permuted to (b2, h_local, d) row order

On-chip per core: qk^T via PE (d-major), v via PE (token-major), scores
computed transposed (keys on partitions), softmax without max-subtraction
(scores are O(1) by construction), denominator via ones-column in V.

Schedule: the PE executes its queue in order, and the attention inner loop
is ACT(exp)-limited (~1310ns/kc vs ~1036ns of PE work), so next-batch QKV
and the first projection wave are issued INTERLEAVED into the attention
kc-loop as fillers.  Input DMA is chunk-ordered so the first matmul starts
~1us in.  Softmax normalization multiplies straight out of PSUM (only the
denominator row is staged to SBUF via the Scalar engine).  Output is bf16
(halves the store DMA); the host pairwise-sum upcasts to f32.
"""

import numpy as np
import ml_dtypes

import concourse.bass as bass
import concourse.mybir as mybir
from concourse import bacc
from concourse.tile import TileContext
from concourse.bass_utils import run_bass_kernel_spmd


N, B, C, H, HD = 1024, 4, 1024, 16, 64
NT = B * N          # 4096 tokens
NCORES = 8
BF = mybir.dt.bfloat16
F32 = mybir.dt.float32
bf16 = ml_dtypes.bfloat16

_NC_CACHE = {}


def build_nc():
    nc = bacc.Bacc()
    xT_e = nc.declare_dram_parameter("xT", [C, NT], BF, isOutput=False)
    wqk_e = nc.declare_dram_parameter("w_qk", [C, 256], BF, isOutput=False)
    wv_e = nc.declare_dram_parameter("w_v", [C, 128], BF, isOutput=False)
    wp_e = nc.declare_dram_parameter("w_p", [512, C], BF, isOutput=False)
    out_e = nc.declare_dram_parameter("out", [N, C], BF, isOutput=True)

    xT_ap = xT_e[:].rearrange("(co p) t -> p co t", p=128)    # [128, 8, 4096]
    wqk_ap = wqk_e[:].rearrange("(co p) m -> p co m", p=128)  # [128, 8, 256]
    wv_ap = wv_e[:].rearrange("(co p) m -> p co m", p=128)    # [128, 8, 128]
    wp_ap = wp_e[:].rearrange("(b2 p) d -> p b2 d", p=128)    # [128, 4, 1024]

    from contextlib import ExitStack
    with TileContext(nc) as tc:
        with ExitStack() as stk:
            cpool = stk.enter_context(tc.tile_pool(name="const", bufs=1))
            epool = stk.enter_context(tc.tile_pool(name="exp", bufs=6))
            spool = stk.enter_context(tc.tile_pool(name="small", bufs=6))
            opool = stk.enter_context(tc.tile_pool(name="outcp", bufs=8))
            dpool = stk.enter_context(
                tc.tile_pool(name="dram", bufs=2, space="DRAM"))
            attn_stk = ExitStack()
            ps_qk = attn_stk.enter_context(
                tc.tile_pool(name="ps_qk", bufs=2, space="PSUM"))
            ps_sT = attn_stk.enter_context(
                tc.tile_pool(name="ps_sT", bufs=2, space="PSUM"))
            ps_av = attn_stk.enter_context(
                tc.tile_pool(name="ps_av", bufs=2, space="PSUM"))
            # ---- persistent SBUF tensors -------------------------------
            xc = [[cpool.tile([128, N], BF, name=f"xc_{b}_{kc}")
                   for kc in range(8)] for b in range(B)]
            wqk = cpool.tile([128, 8, 256], BF)
            wv = cpool.tile([128, 8, 128], BF)
            wp = cpool.tile([128, 4, C], BF)
            q_sb = cpool.tile([128, NT], BF)       # [ (h0|h1) d, token ]
            k_sb = cpool.tile([128, NT], BF)
            # v token-major: [t_in, t_out, hl, (64 dims + ones col)]
            v_sb = cpool.tile([128, 32, 2, 65], BF)
            # projin split by query half so the qt=1 norm writes don't
            # serialize against proj reads of qt=0 tokens
            projin = [cpool.tile([128, B, 512], BF, name=f"projin_{qt}")
                      for qt in range(2)]          # [(hl,d), b2, n-half]

            # DMA order: wqk chunks interleaved with batch-0 x so the first
            # matmul is gated on ~320KB, not 2.5MB.
            for kc in range(8):
                nc.sync.dma_start(out=wqk[:, kc, :], in_=wqk_ap[:, kc, :])
                nc.sync.dma_start(out=xc[0][kc][:],
                                  in_=xT_ap[:, kc, 0:N])
            nc.sync.dma_start(out=wv[:], in_=wv_ap)
            for b in range(1, B):
                for kc in range(8):
                    nc.sync.dma_start(out=xc[b][kc][:],
                                      in_=xT_ap[:, kc, b * N:(b + 1) * N])
            nc.sync.dma_start(out=wp[:], in_=wp_ap)

            nc.vector.memset(v_sb[:, :, :, 64:65], 1.0)
            ones65 = cpool.tile([65, 64], BF)
            nc.vector.memset(ones65[:], 1.0)

            def gen_qk(b):
                """Issue q/k matmuls for batch b, yielding after each PE op."""
                for tc_i in (2 * b, 2 * b + 1):
                    qps = ps_qk.tile([128, 512], F32, tag="qk",
                                     name=f"qps_{tc_i}")
                    kps = ps_qk.tile([128, 512], F32, tag="qk",
                                     name=f"kps_{tc_i}")
                    j = tc_i - 2 * b
                    sl = slice(j * 512, (j + 1) * 512)
                    for kc in range(8):
                        nc.tensor.matmul(qps[:], wqk[:, kc, 0:128],
                                         xc[b][kc][:, sl],
                                         start=(kc == 0), stop=(kc == 7))
                        yield
                        nc.tensor.matmul(kps[:], wqk[:, kc, 128:256],
                                         xc[b][kc][:, sl],
                                         start=(kc == 0), stop=(kc == 7))
                        if kc == 7:
                            # flush casts with the last matmul so consumers
                            # issued right after the final pull see them
                            osl = slice(tc_i * 512, (tc_i + 1) * 512)
                            nc.vector.tensor_copy(out=q_sb[:, osl],
                                                  in_=qps[:])
                            nc.vector.tensor_copy(out=k_sb[:, osl],
                                                  in_=kps[:])
                        yield

            def gen_v(b):
                """Issue v matmuls for batch b, yielding after each PE op."""
                for tt in range(8 * b, 8 * b + 8):
                    vps = ps_qk.tile([128, 2, 64], F32, tag="qk",
                                     name=f"vps_{tt}")
                    tsl = slice((tt - 8 * b) * 128, (tt - 8 * b + 1) * 128)
                    for kc in range(8):
                        nc.tensor.matmul(vps[:, :, :], xc[b][kc][:, tsl],
                                         wv[:, kc, :],
                                         start=(kc == 0), stop=(kc == 7))
                        if kc == 7:
                            nc.vector.tensor_copy(out=v_sb[:, tt, :, 0:64],
                                                  in_=vps[:, :, :])
                        yield

            def gen_proj_wave(nts, use_act=False):
                """Partial projection for token tiles nts; yields per PE op."""
                for nt in nts:
                    pps0 = ps_qk.tile([128, 512], F32, tag="qk",
                                      name=f"pps0_{nt}")
                    pps1 = ps_qk.tile([128, 512], F32, tag="qk",
                                      name=f"pps1_{nt}")
                    pj = projin[nt // 4]
                    nsl = slice((nt % 4) * 128, (nt % 4 + 1) * 128)
                    osl = slice(nt * 128, (nt + 1) * 128)
                    for b2 in range(B):
                        nc.tensor.matmul(pps0[:], pj[:, b2, nsl],
                                         wp[:, b2, 0:512],
                                         start=(b2 == 0), stop=(b2 == 3))
                        yield
                        nc.tensor.matmul(pps1[:], pj[:, b2, nsl],
                                         wp[:, b2, 512:1024],
                                         start=(b2 == 0), stop=(b2 == 3))
                        if b2 == 3:
                            ocp = opool.tile([128, 1024], BF, tag="o",
                                             name=f"ocp_{nt}")
                            if use_act:
                                nc.scalar.copy(out=ocp[:, 0:512], in_=pps0[:])
                            else:
                                nc.vector.tensor_copy(out=ocp[:, 0:512],
                                                      in_=pps0[:])
                            nc.vector.tensor_copy(out=ocp[:, 512:1024],
                                                  in_=pps1[:])
                            nc.sync.dma_start(out=out_e[osl, :], in_=ocp[:])
                        yield

            def pull(gen, n):
                if gen is None:
                    return
                for _ in range(n):
                    if next(gen, "done") == "done":
                        return

            def attn_block(b, qt, filler=None, fills=(6,) * 8):
                q_sl = slice(b * N + qt * 512, b * N + (qt + 1) * 512)
                av0 = ps_av.tile([65, 512], F32, tag="av", name=f"av0_{b}_{qt}")
                av1 = ps_av.tile([65, 512], F32, tag="av", name=f"av1_{b}_{qt}")
                avs = [av0, av1]
                es = []

                def av_mms(kc):
                    e = es[kc]
                    for hl in range(2):
                        nc.tensor.matmul(
                            avs[hl][:],
                            v_sb[:, 8 * b + kc, hl, :],
                            e[:, hl * 512:(hl + 1) * 512],
                            start=(kc == 0), stop=(kc == 7))

                for kc in range(8):
                    k_sl = slice(b * N + kc * 128, b * N + (kc + 1) * 128)
                    sT = ps_sT.tile([128, 1024], F32, tag="sT",
                                    name=f"sT_{b}_{qt}_{kc}")
                    for hl in range(2):
                        nc.tensor.matmul(
                            sT[:, hl * 512:(hl + 1) * 512],
                            k_sb[hl * 64:(hl + 1) * 64, k_sl],
                            q_sb[hl * 64:(hl + 1) * 64, q_sl],
                            start=True, stop=True,
                            tile_position=(hl * 64, 0))
                    e = epool.tile([128, 1024], BF, tag="e",
                                   name=f"e_{b}_{qt}_{kc}")
                    nc.scalar.activation(
                        e[:], sT[:], mybir.ActivationFunctionType.Exp)
                    es.append(e)
                    if kc >= 1:
                        av_mms(kc - 1)
                    pull(filler, fills[kc])
                av_mms(7)
                return avs

            def norm_tail(b, qt, avs):
                """Last-block normalization: no DMA hops on the critical
                path.  Single-partition reciprocal from the PSUM den row
                (slow per element but latency-optimal here), then a K=1
                outer-product matmul broadcasts it across partitions."""
                rcb = spool.tile([65, 1024], BF, tag="rcb", bufs=1,
                                 name="rcb_tail")
                avsb = spool.tile([64, 1024], F32, tag="avsb", bufs=2,
                                  name="avsb_tail")
                for hl in range(2):
                    # av evac on ACT, reciprocal on DVE — they overlap
                    nc.scalar.copy(out=avsb[:, hl * 512:(hl + 1) * 512],
                                   in_=avs[hl][0:64, :])
                    with nc.allow_low_precision(
                            reason="bf16 1/den broadcast; rel-err budget ok"):
                        nc.vector.reciprocal(
                            rcb[64:65, hl * 512:(hl + 1) * 512],
                            avs[hl][64:65, :])
                for hl in range(2):
                    # broadcast lands in the av-tag psum slots (NOT sT/qk
                    # banks, which wave 2's pool reuses — a collision there
                    # would gate wave 2 on this chain)
                    rb_ps = ps_av.tile([64, 512], F32, tag="av",
                                       name=f"rb_ps_{hl}")
                    nc.tensor.matmul(
                        rb_ps[:],
                        ones65[64:65, 0:64],
                        rcb[64:65, hl * 512:(hl + 1) * 512],
                        start=True, stop=True)
                    nc.vector.tensor_mul(
                        projin[qt][hl * 64:(hl + 1) * 64, b, :],
                        avsb[:, hl * 512:(hl + 1) * 512],
                        rb_ps[:])

            def norm_block(b, qt, avs):
                # Evacuate av to SBUF quickly (frees the PSUM slots so the
                # next block's av matmuls aren't blocked on rotation), then
                # den-gather / reciprocal (partition-spread — DVE reciprocal
                # is ~7.7 cy/elem/lane) / DRAM-broadcast / multiply.
                avsb = spool.tile([65, 1024], F32, tag="avsb", bufs=2,
                                  name=f"avsb_{b}_{qt}")
                for hl in range(2):
                    nc.vector.tensor_copy(
                        out=avsb[:, hl * 512:(hl + 1) * 512], in_=avs[hl][:])
                den = spool.tile([16, 64], F32, tag="den", name=f"den_{b}_{qt}")
                nc.gpsimd.dma_start(out=den[:], in_=avsb[64:65, 0:1024],
                                    single_packet=True)
                rcp = spool.tile([16, 64], F32, tag="rcp", name=f"rcp_{b}_{qt}")
                nc.vector.reciprocal(rcp[:], den[:])
                db = dpool.tile([1024], F32, name=f"db_{b}_{qt}")
                nc.gpsimd.dma_start(out=db[:], in_=rcp[:], single_packet=True)
                rb2 = spool.tile([64, 1024], F32, tag="rb", bufs=2,
                                 name=f"rb_{b}_{qt}")
                db_ap = db[:]
                nc.gpsimd.dma_start(
                    out=rb2[:],
                    in_=bass.AP(tensor=db_ap.tensor, offset=db_ap.offset,
                                ap=[[0, 64], [1, 1024]]))
                for hl in range(2):
                    nc.vector.tensor_mul(
                        projin[qt][hl * 64:(hl + 1) * 64, b, :],
                        avsb[0:64, hl * 512:(hl + 1) * 512],
                        rb2[:, hl * 512:(hl + 1) * 512])

            # ---- schedule ---------------------------------------------
            # One global filler stream feeding the attention blocks'
            # PE-idle slots, in dependency-compatible order.  Blocks run
            # 00,01,10,11,20,30,21,31: (2,1) is delayed past (3,0) so the
            # first projection wave (gated on norm(3,0)'s chain, ~10us of
            # DMA latency) lands inside it instead of stalling the tail.
            def chain(*gens):
                for g in gens:
                    yield from g

            gq0 = gen_qk(0)
            pull(gq0, 16)            # token-half 0 of q/k(0): enough to
            # start attention; iter-0 of block (0,0) is front-loaded with
            # the rest of qk(0) — it's DMA-paced anyway.
            stream = chain(gq0, gen_v(0),
                           gen_qk(1), gen_v(1),
                           gen_qk(2), gen_v(2),
                           gen_qk(3), gen_v(3),
                           gen_proj_wave(range(0, 4)))
            blocks = [
                (0, 0, (24, 8, 8, 8, 8, 8, 8, 8)),   # qk0 rest + v0
                (0, 1, (6,) * 8),                    # qk1, v1 ...
                (1, 0, (6,) * 8),
                (1, 1, (6,) * 8),
                (2, 0, (6,) * 8),
                (2, 1, (6,) * 8),                    # qk3 + v3[0:2]
                (3, 0, (6,) * 8),                    # v3 just-in-time
                (3, 1, (4,) * 8),                    # proj wave nt 0-3
            ]
            for b, qt, f in blocks:
                avs = attn_block(b, qt, stream, f)
                if (b, qt) == (3, 1):
                    norm_tail(b, qt, avs)
                else:
                    norm_block(b, qt, avs)
            pull(stream, 10_000)
            attn_stk.close()
            # wave 2 (nt 4..7) on its own 4-slot pool: b2<=2 accumulation
            # starts as soon as the attention banks drain; copies pipeline
            # without slot-rotation waits.
            # wave 2: accumulate b2<=2 for all four token tiles first (not
            # gated on the last norm), then the b2=3 pass + copies + stores.
            with tc.tile_pool(name="ps_p", bufs=3, space="PSUM") as ps_p:
                ppss = []
                for nt in range(4, 8):
                    pps = ps_p.tile([128, 1024], F32, tag="pp",
                                    name=f"pps_{nt}")
                    ppss.append(pps)
                    nsl = slice((nt % 4) * 128, (nt % 4 + 1) * 128)
                    for b2 in range(3):
                        nc.tensor.matmul(pps[:, 0:512],
                                         projin[1][:, b2, nsl],
                                         wp[:, b2, 0:512],
                                         start=(b2 == 0), stop=False)
                        nc.tensor.matmul(pps[:, 512:1024],
                                         projin[1][:, b2, nsl],
                                         wp[:, b2, 512:1024],
                                         start=(b2 == 0), stop=False)
                for nt in range(4, 8):
                    pps = ppss[nt - 4]
                    nsl = slice((nt % 4) * 128, (nt % 4 + 1) * 128)
                    osl = slice(nt * 128, (nt + 1) * 128)
                    nc.tensor.matmul(pps[:, 0:512], projin[1][:, 3, nsl],
                                     wp[:, 3, 0:512],
                                     start=False, stop=True)
                    nc.tensor.matmul(pps[:, 512:1024], projin[1][:, 3, nsl],
                                     wp[:, 3, 512:1024],
                                     start=False, stop=True)
                    ocp = opool.tile([128, 1024], BF, tag="o",
                                     name=f"ocp_{nt}")
                    if nt % 2 == 0:
                        nc.scalar.copy(out=ocp[:], in_=pps[:])
                    else:
                        nc.vector.tensor_copy(out=ocp[:], in_=pps[:])
                    nc.sync.dma_start(out=out_e[osl, :], in_=ocp[:])

    nc.compile()
    return nc


def _prep_core(i, xT, w_qkv, w_proj):
    """Per-core input shards (host-side layout absorption)."""
    h0 = 2 * i
    rows = np.concatenate([np.arange(h0 * HD, (h0 + 1) * HD),
                           np.arange((h0 + 1) * HD, (h0 + 2) * HD)])
    w_qk = np.concatenate([w_qkv[rows] * 0.125, w_qkv[C + rows]], axis=0).T
    w_v = w_qkv[2 * C + rows].T
    hh = np.array([h0, h0 + 1])
    cg = ((hh % 4)[None, :, None] * 256
          + np.arange(B)[:, None, None] * 64
          + np.arange(HD)[None, None, :])          # [b2, hl, d]
    w_p = w_proj[:, cg.reshape(-1)].T              # [512, 1024]
    return {
        "xT": xT,
        "w_qk": np.ascontiguousarray(w_qk, dtype=bf16),
        "w_v": np.ascontiguousarray(w_v, dtype=bf16),
        "w_p": np.ascontiguousarray(w_p, dtype=bf16),
    }


def _run(inputs, trace=False, **kw):
    x = np.asarray(inputs["x"], dtype=np.float32)
    w_qkv = np.asarray(inputs["w_qkv"], dtype=np.float32)
    w_proj = np.asarray(inputs["w_proj"], dtype=np.float32)
    b_proj = np.asarray(inputs["b_proj"], dtype=np.float32)

    if "nc" not in _NC_CACHE:
        _NC_CACHE["nc"] = build_nc()
    nc = _NC_CACHE["nc"]

    xT = np.ascontiguousarray(
        x.transpose(2, 1, 0).reshape(C, NT), dtype=bf16)
    in_maps = [_prep_core(i, xT, w_qkv, w_proj) for i in range(NCORES)]
    res = run_bass_kernel_spmd(nc, in_maps, core_ids=list(range(NCORES)),
                               trace=trace, **kw)
    out = np.empty((N, B, C), np.float32)
    for j in range(4):
        out[:, j, :] = (res.results[2 * j]["out"].astype(np.float32)
                        + res.results[2 * j + 1]["out"].astype(np.float32)
                        + b_proj)
    return out, res


def kernel(**inputs) -> np.ndarray:
    out, _ = _run(inputs, trace=False)
    return out
